# revision 14
# baseline (speedup 1.0000x reference)
"""Trainium2 Bass kernel for nn_Decoder (MLP -> inverse token embedding ->
overlap-add -> channel-merge conv), data-parallel over batch on 8 NeuronCores.

Self-contained: hardcodes shapes; host-side numpy folds everything after the
first Linear+ReLU into per-channel fused matrices G (W2 -> Winv -> overlap-add
normalization -> 3-tap channel conv), so the device pipeline is:

    x[tok,E] --PE transpose--> xT[E,tok] --matmul W1T--> h[Hc,tok] in PSUM
    --ACT/DVE relu+bias--> hT in SBUF --matmul G (accum over c,Hc)--> v[66,tok]
    --PE transpose--> vT[b,66] --strided DVE adds (overlap-add)--> y[b,1056]

Sharding: batch 1024 -> 8 cores x 128.
"""

import numpy as np

import concourse.bacc as bacc
import concourse.mybir as mybir
from concourse.bass_utils import run_bass_kernel_spmd
from concourse.tile import TileContext

# problem shapes (hardcoded per contract)
B, C, T, E, H = 1024, 8, 32, 128, 256
SEG_LEN, SIG_LEN, NUM_SEG, STEP = 64, 1056, 32, 32
N_CORES = 8
BL = B // N_CORES          # local batch per core = 128
HC = H // 128              # H chunks = 2
TC = 8                     # t-chunks
TL = T // TC               # t per chunk = 4
FD = mybir.dt.float32

_CACHE = {}


def _host_prep(W1, b1, W2, b2, Winv, binv, Wconv, bconv):
    """Fold W2/Winv/normalization/conv into G [3var][C][H,66] and bias B[1056]."""
    counter = np.zeros(SIG_LEN, np.float64)
    for t in range(NUM_SEG):
        counter[t * STEP: t * STEP + SEG_LEN] += 1.0
    n = 1.0 / counter

    F = Winv.astype(np.float64) @ W2.astype(np.float64)          # [64, H]
    binv2 = Winv.astype(np.float64) @ b2.astype(np.float64) + binv.astype(np.float64)
    Wc = Wconv[0].astype(np.float64)                             # [C, 3]

    def n_of(var, s):
        if var == 0:
            return n[s]
        if var == 2:
            return n[992 + s]
        return 0.5

    G = np.zeros((3, C, H, 66), np.float64)
    for var in range(3):
        for c in range(C):
            for m_idx in range(66):
                for k in range(3):
                    s = m_idx + k - 2
                    if 0 <= s < SEG_LEN:
                        G[var, c, :, m_idx] += Wc[c, k] * n_of(var, s) * F[s, :]

    sig_b = np.zeros(SIG_LEN, np.float64)
    for t in range(NUM_SEG):
        sig_b[t * STEP: t * STEP + SEG_LEN] += binv2
    sig_b *= n
    Bvec = np.full(SIG_LEN, float(np.asarray(bconv).reshape(-1)[0]), np.float64)
    q = np.arange(SIG_LEN)
    for k in range(3):
        qq = q + k - 1
        valid = (qq >= 0) & (qq < SIG_LEN)
        for c in range(C):
            Bvec[valid] += Wc[c, k] * sig_b[qq[valid]]
    return G.astype(np.float32), Bvec.astype(np.float32)


def _g_col(hc, c, var):
    """Column offset of G slice (hc, c, var) inside g_sb [128, 2*8*3*66]."""
    return ((hc * C + c) * 3 + var) * 66


def _build_bass(debug=False):
    nc = bacc.Bacc("TRN2")

    x = nc.dram_tensor("x", [BL, C, T, E], FD, kind="ExternalInput")
    w1t = nc.dram_tensor("w1t", [E, H], FD, kind="ExternalInput")
    b1c = nc.dram_tensor("b1c", [128, HC], FD, kind="ExternalInput")
    g = nc.dram_tensor("g", [128, HC * C * 3 * 66], FD, kind="ExternalInput")
    brep = nc.dram_tensor("brep", [BL, SIG_LEN], FD, kind="ExternalInput")
    ident = nc.dram_tensor("ident", [128, 128], FD, kind="ExternalInput")
    y = nc.dram_tensor("y", [BL, SIG_LEN], FD, kind="ExternalOutput")
    if debug:
        dbg_xt = nc.dram_tensor("dbg_xt", [128, TL * 128], FD, kind="ExternalOutput")
        dbg_ht = nc.dram_tensor("dbg_ht", [HC, 128, C * TL * 128], FD,
                                kind="ExternalOutput")
        dbg_v = nc.dram_tensor("dbg_v", [BL, T * 66], FD, kind="ExternalOutput")

    with TileContext(nc) as tc:
        with (
            tc.tile_pool(name="consts", bufs=1) as consts,
            tc.tile_pool(name="xin", bufs=6) as xin_pool,
            tc.tile_pool(name="xt", bufs=10) as xt_pool,
            tc.tile_pool(name="ht", bufs=4) as ht_pool,
            tc.tile_pool(name="vsb", bufs=2) as vsb_pool,
            tc.tile_pool(name="big", bufs=1) as big_pool,
            tc.tile_pool(name="pe_out", bufs=2, space="PSUM") as peout_pool,
            tc.tile_pool(name="h_ps", bufs=4, space="PSUM") as hps_pool,
            tc.tile_pool(name="v_ps", bufs=2, space="PSUM") as vps_pool,
        ):
            w1t_sb = consts.tile([E, H], FD)
            nc.sync.dma_start(out=w1t_sb[:], in_=w1t[:])
            b1c_sb = consts.tile([128, HC], FD)
            nc.sync.dma_start(out=b1c_sb[:], in_=b1c[:])
            g_sb = consts.tile([128, HC * C * 3 * 66], FD)
            nc.sync.dma_start(out=g_sb[:], in_=g[:])
            ident_sb = consts.tile([128, 128], FD)
            nc.sync.dma_start(out=ident_sb[:], in_=ident[:])
            brep_sb = big_pool.tile([BL, SIG_LEN], FD)
            nc.sync.dma_start(out=brep_sb[:], in_=brep[:])

            V_sb = big_pool.tile([BL, T * 66], FD)      # v transposed: [b, t*66+m]
            y_sb = big_pool.tile([BL, SIG_LEN], FD)

            # software pipeline: fused stage runs one t-chunk behind MLP1
            ht_tiles = {}

            def chunk_ranges(tcix):
                # column ranges with uniform G variant; cols = tl*128 + b
                if tcix == 0:
                    return [(0, 128, 0), (128, 512, 1)]       # t=0 -> var 0
                if tcix == TC - 1:
                    return [(0, 384, 1), (384, 512, 2)]       # t=31 -> var 2
                return [(0, 512, 1)]

            def emit_loads_transposes(tcix):
                xt_list = []
                for c in range(C):
                    # load x block: [b=128 part, (tl, e)]
                    xtile = xin_pool.tile([BL, TL, E], FD, tag="xin")
                    nc.sync.dma_start(
                        out=xtile[:],
                        in_=x[:, c, tcix * TL:(tcix + 1) * TL, :],
                    )
                    # PE transpose each [b, e] slice -> xT [e, tl*128 + b]
                    xt_ps = peout_pool.tile([128, TL * 128], FD, tag="pe_out")
                    for tl in range(TL):
                        nc.tensor.transpose(
                            xt_ps[:, tl * 128:(tl + 1) * 128],
                            xtile[:, tl, :],
                            ident_sb[:],
                        )
                    xt_sb = xt_pool.tile([128, TL * 128], FD, tag="xt")
                    nc.scalar.copy(out=xt_sb[:], in_=xt_ps[:])
                    xt_list.append(xt_sb)
                    if debug and tcix == 0 and c == 0:
                        nc.sync.dma_start(out=dbg_xt[:], in_=xt_sb[:])
                return xt_list

            def emit_mlp1(tcix, xt_list, c):
                ht = ht_tiles[tcix]
                xt_sb = xt_list[c]
                for hc in range(HC):
                    h_ps = hps_pool.tile([128, TL * 128], FD, tag="h_ps")
                    nc.tensor.matmul(
                        h_ps[:],
                        w1t_sb[:, hc * 128:(hc + 1) * 128],
                        xt_sb[:],
                        start=True, stop=True,
                    )
                    # relu + bias -> hT slice; alternate ACT/DVE engines
                    dst = ht[hc][:, c * TL * 128:(c + 1) * TL * 128]
                    if (c * HC + hc) % 16 < 6:
                        nc.scalar.activation(
                            dst, h_ps[:],
                            mybir.ActivationFunctionType.Relu,
                            bias=b1c_sb[:, hc:hc + 1], scale=1.0,
                        )
                    else:
                        nc.vector.tensor_scalar(
                            dst, h_ps[:],
                            b1c_sb[:, hc:hc + 1], 0.0,
                            mybir.AluOpType.add, mybir.AluOpType.max,
                        )

            def emit_fused(tcix, v_tiles, c):
                """fused G matmuls for channel c accumulating into v_tiles."""
                ht = ht_tiles[tcix]
                for (lo, hi, var, v_ps) in v_tiles:
                    for hc in range(HC):
                        i = c * HC + hc
                        nc.tensor.matmul(
                            v_ps[:, lo:hi],
                            g_sb[:, _g_col(hc, c, var):_g_col(hc, c, var) + 66],
                            ht[hc][:, c * 512 + lo:c * 512 + hi],
                            start=(i == 0), stop=(i == C * HC - 1),
                        )

            def emit_vtrans(tcix, v_tiles):
                """copy v psum -> sbuf, PE-transpose per t into V_sb."""
                if debug and tcix == 0:
                    for hc in range(HC):
                        nc.sync.dma_start(out=dbg_ht[hc], in_=ht_tiles[0][hc][:])
                del ht_tiles[tcix]
                v_sb = vsb_pool.tile([66, 512], FD, tag="v_sb")
                for (lo, hi, var, v_ps) in v_tiles:
                    nc.vector.tensor_copy(out=v_sb[:, lo:hi], in_=v_ps[:, lo:hi])
                for tl in range(TL):
                    t = tcix * TL + tl
                    vt_ps = peout_pool.tile([128, 66], FD, tag="pe_out")
                    nc.tensor.transpose(
                        vt_ps[:],
                        v_sb[:, tl * 128:(tl + 1) * 128],
                        ident_sb[0:66, 0:66],
                    )
                    nc.vector.tensor_copy(
                        out=V_sb[:, t * 66:(t + 1) * 66], in_=vt_ps[:])

            prev = None          # (tcix, v_tiles) of the chunk awaiting fused stage
            for tcix in range(TC):
                ht_tiles[tcix] = [
                    ht_pool.tile([128, C * TL * 128], FD, tag=f"ht{hc}", name=f"ht_{tcix}_{hc}")
                    for hc in range(HC)]
                xt_list = emit_loads_transposes(tcix)
                # interleave: MLP1(tcix, c) with fused(tcix-1, c) so PE always
                # has matmul work while relu copies drain PSUM
                for c in range(C):
                    emit_mlp1(tcix, xt_list, c)
                    if prev is not None:
                        emit_fused(prev[0], prev[1], c)
                if prev is not None:
                    emit_vtrans(prev[0], prev[1])
                v_tiles = [
                    (lo, hi, var, vps_pool.tile([66, 512], FD, tag="v_ps", name=f"v_ps_{tcix}_{lo}"))
                    for (lo, hi, var) in chunk_ranges(tcix)]
                prev = (tcix, v_tiles)
            for c in range(C):
                emit_fused(prev[0], prev[1], c)
            emit_vtrans(prev[0], prev[1])
            if debug:
                nc.sync.dma_start(out=dbg_v[:], in_=V_sb[:])

            # overlap-add assembly (all on partition b, strided free-dim APs)
            V3 = V_sb[:].rearrange("b (t m) -> b t m", m=66)
            Y3 = y_sb[:].rearrange("b (j r) -> b j r", r=32)
            # y[:, 0:1024] = brep[:, 0:1024] + v[:, j, r+1]
            nc.vector.tensor_add(
                out=Y3[:, 0:32, :], in0=V3[:, :, 1:33],
                in1=brep_sb[:].rearrange("b (j r) -> b j r", r=32)[:, 0:32, :])
            # y[:, 1024:1056] = brep  (v[31] term arrives via the += below)
            nc.vector.tensor_copy(
                out=y_sb[:, 1024:1056], in_=brep_sb[:, 1024:1056])
            # y[:, 32:1056] += v[:, j-1, r+33]
            nc.vector.tensor_add(
                out=Y3[:, 1:33, :], in0=Y3[:, 1:33, :], in1=V3[:, :, 33:65])
            # y[:, 64::32] += v[:, j-2, 65]   (j=2..32)
            nc.vector.tensor_add(
                out=Y3[:, 2:33, 0], in0=Y3[:, 2:33, 0], in1=V3[:, 0:31, 65])
            # y[:, 31:1024:32] += v[:, j+1, 0]  (j=0..30)
            nc.vector.tensor_add(
                out=Y3[:, 0:31, 31], in0=Y3[:, 0:31, 31], in1=V3[:, 1:32, 0])

            nc.sync.dma_start(out=y[:], in_=y_sb[:])

    nc.finalize()
    return nc


def kernel(**inputs) -> np.ndarray:
    x = np.ascontiguousarray(np.asarray(inputs["encoder_output"], dtype=np.float32))
    W1 = np.asarray(inputs["W1"], np.float32)
    b1 = np.asarray(inputs["b1"], np.float32)

    G, Bvec = _host_prep(
        inputs["W1"], inputs["b1"], inputs["W2"], inputs["b2"],
        inputs["Winv"], inputs["binv"], inputs["Wconv"], inputs["bconv"])

    # pack G -> [128, HC*C*3*66]: g_sb[p, _g_col(hc,c,var)+m] = G[var, c, hc*128+p, m]
    g_pack = np.zeros((128, HC * C * 3 * 66), np.float32)
    for hc in range(HC):
        for c in range(C):
            for var in range(3):
                col = _g_col(hc, c, var)
                g_pack[:, col:col + 66] = G[var, c, hc * 128:(hc + 1) * 128, :]

    w1t = np.ascontiguousarray(W1.T)                        # [E, H]
    b1c = np.ascontiguousarray(b1.reshape(HC, 128).T)       # [128, HC]
    brep = np.ascontiguousarray(np.broadcast_to(Bvec, (BL, SIG_LEN)))
    ident = np.eye(128, dtype=np.float32)

    if "nc" not in _CACHE:
        _CACHE["nc"] = _build_bass()
    nc = _CACHE["nc"]

    shards = x.reshape(N_CORES, BL, C, T, E)
    in_maps = [
        {
            "x": np.ascontiguousarray(shards[i]),
            "w1t": w1t, "b1c": b1c, "g": g_pack,
            "brep": brep, "ident": ident,
        }
        for i in range(N_CORES)
    ]
    res = run_bass_kernel_spmd(nc, in_maps, core_ids=list(range(N_CORES)))
    _CACHE["last_result"] = res
    y = np.concatenate([r["y"] for r in res.results], axis=0)   # [B, 1056]
    return y.reshape(B, 1, SIG_LEN).astype(np.float32)


if __name__ == "__main__":
    rng = np.random.default_rng(0)
    ins = {
        "encoder_output": rng.standard_normal((B, C, T, E), dtype=np.float32),
        "W1": rng.standard_normal((H, E), dtype=np.float32) / np.sqrt(E),
        "b1": rng.standard_normal((H,), dtype=np.float32) / np.sqrt(E),
        "W2": rng.standard_normal((E, H), dtype=np.float32) / np.sqrt(H),
        "b2": rng.standard_normal((E,), dtype=np.float32) / np.sqrt(H),
        "Winv": rng.standard_normal((SEG_LEN, E), dtype=np.float32) / np.sqrt(E),
        "binv": rng.standard_normal((SEG_LEN,), dtype=np.float32) / np.sqrt(E),
        "Wconv": rng.standard_normal((1, C, 3), dtype=np.float32) / np.sqrt(C * 3),
        "bconv": rng.standard_normal((1,), dtype=np.float32) / np.sqrt(C * 3),
    }
    out = kernel(**ins)
    print("kernel output", out.shape, out.dtype)
